# revision 10
# baseline (speedup 1.0000x reference)
"""Trainium2 Bass kernel for DemoGraphNet (2-layer GCN + mean-pool + MLP head).

Self-contained: hardcodes problem shapes and the 8-core sharding strategy.

Strategy
--------
Nodes are partitioned contiguously across the 8 cores (12500 each); every edge
is processed on the core owning its dst node (self-loop edges appended).  The
symmetric normalization is folded into the data path so per-edge work is pure
gather+matmul:

    table[n]  = inv_sqrt[n] * (h @ W)[n]          (built shard-wise, AllGathered, bf16)
    S[e, j]   = (dst_e == j) * inv_sqrt[dst_e]    (one-hot built by DVE iota-compare)
    agg[c, j] = sum_e table[src_e][c] * S[e, j]   (TensorE, PSUM-accumulated per window)

Edges are bucketed by (dst-window of 128 nodes, src-block of <=32768 table rows
for int16 gather indices), padded to 128-edge chunks with per-bucket quotas
maxed across cores so a single SPMD program serves all 8 cores.  dma_gather
(SWDGE) fetches table rows into [edge x chan] SBUF tiles.  Mean-pooling is a
batch-id one-hot matmul fused into layer-2 window evacuation (via a TensorE
transpose); per-graph sums are AllReduced and the tiny MLP head runs
replicated on every core.
"""

import math
import os
import sys

sys.path.insert(0, "/opt/trn_rl_repo")

import numpy as np
import ml_dtypes

import concourse.bass as bass
import concourse.mybir as mybir
import concourse.tile as tile
import concourse.bacc as bacc
from concourse import library_config
from concourse.bass_utils import run_bass_kernel_spmd

BF16 = ml_dtypes.bfloat16
F32 = np.float32


class Cfg:
    def __init__(self, n_nodes, n_graphs, st_w, n_cores=8, hid=128, out_c=8,
                 blk_rows=None):
        assert n_nodes % n_cores == 0
        self.N = n_nodes
        self.G = n_graphs
        self.C = n_cores
        self.HID = hid
        self.OUT = out_c
        self.NPC = n_nodes // n_cores          # nodes per core
        self.WIN = 128                          # dst window width
        self.NW = math.ceil(self.NPC / 128)     # windows per core
        self.PAD_NPC = self.NW * 128
        self.ST_W = st_w                        # windows per supertile
        self.NST = math.ceil(self.NW / st_w)
        self.TBL_N = n_cores * self.PAD_NPC     # padded table rows
        if blk_rows is None:
            self.NBLK = max(1, math.ceil(self.TBL_N / 32768))
            self.BLKROWS = math.ceil(self.TBL_N / self.NBLK / 128) * 128
        else:
            self.BLKROWS = blk_rows
            self.NBLK = math.ceil(self.TBL_N / blk_rows)


CFG = Cfg(n_nodes=100000, n_graphs=256, st_w=7)


# ----------------------------------------------------------------- host prep
def _host_prep(cfg, x, edge_index, batch):
    """Shard + marshal inputs. Index bookkeeping only (sorting, bucketing,
    counting, layout); all FLOPs on feature data happen on device."""
    N, C = cfg.N, cfg.C
    NPC, WIN, NW, ST_W, NST = cfg.NPC, cfg.WIN, cfg.NW, cfg.ST_W, cfg.NST
    NBLK, BLKROWS, PAD_NPC = cfg.NBLK, cfg.BLKROWS, cfg.PAD_NPC

    src = np.asarray(edge_index[0], dtype=np.int64)
    dst = np.asarray(edge_index[1], dtype=np.int64)
    batch = np.asarray(batch, dtype=np.int64)
    x = np.asarray(x, dtype=np.float32)

    deg = (np.bincount(dst, minlength=N) + 1).astype(np.float64)

    # append self-loop edges (the +I in A_hat)
    loops = np.arange(N, dtype=np.int64)
    src_all = np.concatenate([src, loops])
    dst_all = np.concatenate([dst, loops])

    core = dst_all // NPC
    tblrow = (src_all // NPC) * PAD_NPC + (src_all % NPC)
    blk = tblrow // BLKROWS
    w_loc = (dst_all % NPC) // WIN
    # bucket counts per (core, window, block)
    key = (core * NW + w_loc) * NBLK + blk
    counts = np.bincount(key, minlength=C * NW * NBLK).reshape(C, NW, NBLK)
    quota = counts.max(axis=0)
    quota = ((quota + 127) // 128) * 128 * (quota > 0)       # [NW, NBLK]

    # bucket bases in the padded edge stream, ordered (st, blk, w)
    bucket_base = np.zeros((NW, NBLK), dtype=np.int64)
    seg_off = np.zeros((NST, NBLK), dtype=np.int64)
    seg_len = np.zeros((NST, NBLK), dtype=np.int64)
    pos = 0
    for st in range(NST):
        ws = range(st * ST_W, min((st + 1) * ST_W, NW))
        for b in range(NBLK):
            seg_off[st, b] = pos
            for w in ws:
                bucket_base[w, b] = pos
                pos += quota[w, b]
            seg_len[st, b] = pos - seg_off[st, b]
    EP = pos
    NCH = EP // 128

    # chunk metadata (core-uniform): global chunk gci -> window; processing
    # order per supertile is window-major
    chunk_win = np.zeros(NCH, dtype=np.int64)
    for w in range(NW):
        for b in range(NBLK):
            c0 = bucket_base[w, b] // 128
            chunk_win[c0:c0 + quota[w, b] // 128] = w
    st_chunks = []   # per st: list of (gci, w, start, stop)
    for st in range(NST):
        ws = range(st * ST_W, min((st + 1) * ST_W, NW))
        entries = []
        for w in ws:
            gcis = []
            for b in range(NBLK):
                c0 = bucket_base[w, b] // 128
                gcis.extend(range(c0, c0 + quota[w, b] // 128))
            for i, gci in enumerate(gcis):
                entries.append((gci, w, i == 0, i == len(gcis) - 1))
        st_chunks.append(entries)
    st_base = [min(e[0] for e in st_chunks[st]) if st_chunks[st] else 0
               for st in range(NST)]
    st_nch = [len(st_chunks[st]) for st in range(NST)]

    cnt = np.bincount(batch, minlength=cfg.G).astype(np.float32)

    in_maps = []
    okey = ((w_loc // ST_W) * NBLK + blk) * NW + w_loc
    for c in range(C):
        sel = np.nonzero(core == c)[0]
        e_okey = okey[sel]
        e_row = tblrow[sel]
        order = np.lexsort((e_row, e_okey))
        sel = sel[order]
        e_okey = e_okey[order]
        # rank within bucket
        bkey = (w_loc[sel] * NBLK + blk[sel])
        change = np.ones(len(sel), dtype=bool)
        change[1:] = bkey[1:] != bkey[:-1]
        gstart = np.maximum.accumulate(np.where(change, np.arange(len(sel)), 0))
        rank = np.arange(len(sel)) - gstart
        posn = bucket_base[w_loc[sel], blk[sel]] + rank

        e_idx16 = np.zeros(EP, dtype=np.int16)
        e_dst = np.full(EP, -1.0, dtype=np.float32)
        e_deg = np.ones(EP, dtype=np.float32)
        e_idx16[posn] = (tblrow[sel] - blk[sel] * BLKROWS).astype(np.int16)
        e_dst[posn] = ((dst_all[sel] % NPC) % WIN).astype(np.float32)
        e_deg[posn] = deg[dst_all[sel]].astype(np.float32)

        idx_img = np.tile(e_idx16.reshape(-1, 16).T, (8, 1)).copy()
        dst_col = e_dst.reshape(NCH, 128).T.copy()
        deg_col = e_deg.reshape(NCH, 128).T.copy()

        lo, hi = c * NPC, (c + 1) * NPC
        x_pad = np.zeros((PAD_NPC, cfg.HID), dtype=np.float32)
        x_pad[:NPC] = x[lo:hi]
        xc = np.ascontiguousarray(x_pad.reshape(NW, 128, cfg.HID).transpose(0, 2, 1))

        deg_own = np.ones(PAD_NPC, dtype=np.float32)
        deg_own[:NPC] = deg[lo:hi]
        deg_pp = deg_own.reshape(NW, 128).T.copy()

        batch_own = np.full(PAD_NPC, -1.0, dtype=np.float32)
        batch_own[:NPC] = batch[lo:hi].astype(np.float32)
        batch_pp = batch_own.reshape(NW, 128).T.copy()

        in_maps.append({
            "xc": xc, "idx": idx_img, "dstc": dst_col, "degc": deg_col,
            "degpp": deg_pp, "batchpp": batch_pp,
            "iota128": np.broadcast_to(
                np.arange(128, dtype=np.float32), (128, 128)).astype(BF16).copy(),
            "iotag": np.broadcast_to(
                np.arange(cfg.G, dtype=np.float32), (128, cfg.G)).astype(BF16).copy(),
            "cnt": cnt[None, :].copy(),
            "ones1": np.ones((1, 128), dtype=np.float32),
        })

    meta = dict(EP=EP, NCH=NCH, seg_off=seg_off, seg_len=seg_len,
                st_chunks=st_chunks, st_base=st_base, st_nch=st_nch)
    return in_maps, meta


def _add_weights(cfg, in_maps, W1, b1, W2, b2, Wh1, bh1, Wh2, bh2):
    wts = {
        "W1": np.asarray(W1, F32), "b1": np.asarray(b1, F32).reshape(-1, 1),
        "W2": np.asarray(W2, F32), "b2": np.asarray(b2, F32).reshape(-1, 1),
        "Wh1": np.asarray(Wh1, F32), "bh1": np.asarray(bh1, F32).reshape(-1, 1),
        "Wh2": np.asarray(Wh2, F32), "bh2": np.asarray(bh2, F32).reshape(-1, 1),
    }
    for m in in_maps:
        m.update(wts)


# ------------------------------------------------------------- program build
def _build(cfg, meta):
    NW, ST_W, NST, NBLK = cfg.NW, cfg.ST_W, cfg.NST, cfg.NBLK
    NCH, EP = meta["NCH"], meta["EP"]
    HID, G = cfg.HID, cfg.G
    bf = mybir.dt.bfloat16
    f32 = mybir.dt.float32

    no_coll = os.environ.get("GNN_NO_COLL") == "1"
    no_gather = os.environ.get("GNN_NO_GATHER") == "1"
    nc = bacc.Bacc("TRN2", target_bir_lowering=False, debug=False,
                   num_devices=cfg.C)
    P = {}
    def param(name, shape, dt=f32):
        P[name] = nc.declare_dram_parameter(name, list(shape), dt, isOutput=False)
        return P[name]

    param("xc", [NW, 128, HID])
    param("idx", [128, EP // 16], mybir.dt.int16)
    param("dstc", [128, NCH]); param("degc", [128, NCH])
    param("degpp", [128, NW]); param("batchpp", [128, NW])
    param("iota128", [128, 128], bf); param("iotag", [128, G], bf)
    param("cnt", [1, G]); param("ones1", [1, 128])
    param("W1", [HID, HID]); param("b1", [HID, 1])
    param("W2", [HID, HID]); param("b2", [HID, 1])
    param("Wh1", [HID, HID]); param("bh1", [HID, 1])
    param("Wh2", [HID, cfg.OUT]); param("bh2", [cfg.OUT, 1])
    t_out = nc.declare_dram_parameter("out", [cfg.OUT, G], f32, isOutput=True)

    replica = [list(range(cfg.C))]
    max_nch = max(meta["st_nch"]) if meta["st_nch"] else 1

    from concourse.masks import make_identity

    with tile.TileContext(nc) as tc:
        with (
            tc.tile_pool(name="const", bufs=1) as cp,
            tc.tile_pool(name="xchunk", bufs=3) as xp,
            tc.tile_pool(name="mtiles", bufs=2) as mp,
            tc.tile_pool(name="stiles", bufs=6) as sp,
            tc.tile_pool(name="evac", bufs=4) as ep,
            tc.tile_pool(name="psw", bufs=3, space="PSUM") as psw,
            tc.tile_pool(name="psa", bufs=3, space="PSUM") as psa,
            tc.tile_pool(name="psg", bufs=1, space="PSUM") as psg,
            tc.tile_pool(name="dram", bufs=1, space="DRAM") as dp,
        ):
            nc.gpsimd.load_library(library_config.mlp)

            # ---- constants / metadata to SBUF
            def load(name, shape, dt=f32):
                t = cp.tile(list(shape), dt, tag=f"c_{name}", name=f"c_{name}")
                nc.sync.dma_start(t[:], P[name][:])
                return t
            idx_sb = load("idx", [128, EP // 16], mybir.dt.int16)
            dst_sb = load("dstc", [128, NCH])
            deg_sb = load("degc", [128, NCH])
            degpp_sb = load("degpp", [128, NW])
            batch_sb = load("batchpp", [128, NW])
            iota_sb = load("iota128", [128, 128], bf)
            iotag_sb = load("iotag", [128, G], bf)
            cnt_sb = load("cnt", [1, G])
            ones1_sb = load("ones1", [1, 128])
            W1_sb = load("W1", [HID, HID]); b1_sb = load("b1", [HID, 1])
            W2_sb = load("W2", [HID, HID]); b2_sb = load("b2", [HID, 1])
            Wh1_sb = load("Wh1", [HID, HID]); bh1_sb = load("bh1", [HID, 1])
            Wh2_sb = load("Wh2", [HID, cfg.OUT]); bh2_sb = load("bh2", [cfg.OUT, 1])

            ident = cp.tile([128, 128], f32, tag="c_ident")
            make_identity(nc, ident[:])

            # inv-sqrt factors: 1/x then sqrt (ACT Rsqrt is banned for accuracy)
            inve_sb = cp.tile([128, NCH], f32, tag="c_inve")
            nc.vector.reciprocal(inve_sb[:], deg_sb[:])
            nc.scalar.activation(inve_sb[:], inve_sb[:],
                                 mybir.ActivationFunctionType.Sqrt)
            invs_sb = cp.tile([128, NW], f32, tag="c_invs")
            nc.vector.reciprocal(invs_sb[:], degpp_sb[:])
            nc.scalar.activation(invs_sb[:], invs_sb[:],
                                 mybir.ActivationFunctionType.Sqrt)

            # node tables (padded global rows), bf16.  Raw dram tensors,
            # manually registered in the DGE table: Tile's symbolic lowering
            # of InstDMAGatherAnt drops the dge_table registration that
            # SWDGE-generated descriptors need for scratchpad relocation
            # (device crash otherwise).
            tbl_own = [nc.dram_tensor(f"tblown{i}", [cfg.PAD_NPC, HID], bf)
                       for i in range(2)]
            tbl_full = [nc.dram_tensor(f"tblfull{i}", [cfg.TBL_N, HID], bf)
                        for i in range(2)]
            for t in tbl_full:
                mloc = nc.lookup_mloc(t)
                if mloc.table_entry_id is None:
                    mloc.table_entry_id = len(nc.dge_table) + 1
                    nc.dge_table.append(mloc.name)

            # ---- phase A (layer 1): table1 = invs * (x @ W1), node-major
            for k in range(NW):
                xk = xp.tile([128, 128], f32, tag="xk")
                nc.sync.dma_start(xk[:], P["xc"][k])
                ps = psa.tile([128, HID], f32, space="PSUM", tag="a")
                nc.tensor.matmul(out=ps[:], lhsT=xk[:], rhs=W1_sb[:],
                                 start=True, stop=True)
                tw = ep.tile([128, HID], bf, tag="tw")
                nc.vector.tensor_scalar(out=tw[:], in0=ps[:],
                                        scalar1=invs_sb[:, k:k + 1], scalar2=None,
                                        op0=mybir.AluOpType.mult)
                nc.sync.dma_start(tbl_own[0][k * 128:(k + 1) * 128, :], tw[:])

            if no_coll:
                nc.sync.dma_start(tbl_full[0][:cfg.PAD_NPC, :], tbl_own[0][:])
            else:
                nc.gpsimd.collective_compute(
                    "AllGather", mybir.AluOpType.bypass,
                    ins=[tbl_own[0][:]], outs=[tbl_full[0][:]],
                    replica_groups=replica)

            ps_pool = psg.tile([128, G], f32, space="PSUM", tag="g")

            # ---- aggregation sweep (shared for both layers)
            def agg_layer(layer):
                tbl = tbl_full[layer]
                for st in range(NST):
                    entries = meta["st_chunks"][st]
                    if not entries:
                        continue
                    base = meta["st_base"][st]
                    m_sb = mp.tile([128, max_nch, 128], bf, tag="mtile")
                    for b in range(NBLK):
                        off = int(meta["seg_off"][st, b])
                        ln = int(meta["seg_len"][st, b])
                        if ln == 0:
                            continue
                        c0 = off // 128 - base
                        if no_gather:
                            nc.gpsimd.memset(m_sb[:, c0:c0 + ln // 128, :], 0.5)
                        else:
                            nc.gpsimd.dma_gather(
                                m_sb[:, c0:c0 + ln // 128, :],
                                tbl[b * cfg.BLKROWS:
                                    min((b + 1) * cfg.BLKROWS, cfg.TBL_N), :],
                                idx_sb[:, off // 16: off // 16 + ln // 16],
                                ln, ln, HID,
                                single_packet=False)
                    lastw = {}
                    for gci, w, first, last in entries:
                        if first:
                            lastw[w] = psw.tile([128, 128], f32, space="PSUM",
                                                tag="win", name=f"win_{layer}_{w}")
                        s_sb = sp.tile([128, 128], bf, tag="s")
                        eng = nc.vector  # TODO: alternate with gpsimd once validated
                        eng.tensor_scalar(
                            out=s_sb[:], in0=iota_sb[:],
                            scalar1=dst_sb[:, gci:gci + 1],
                            scalar2=inve_sb[:, gci:gci + 1],
                            op0=mybir.AluOpType.is_equal,
                            op1=mybir.AluOpType.mult)
                        nc.tensor.matmul(out=lastw[w][:],
                                         lhsT=m_sb[:, gci - base, :], rhs=s_sb[:],
                                         start=first, stop=last)
                        if last:
                            finish_window(layer, w, lastw[w])

            def finish_window(layer, w, ps_w):
                if layer == 0:
                    # h1 window + fused table2 build
                    h1w = ep.tile([128, 128], f32, tag="h1w")
                    nc.scalar.activation(h1w[:], ps_w[:],
                                         mybir.ActivationFunctionType.Relu,
                                         bias=b1_sb[:, 0:1])
                    ps2 = psa.tile([128, HID], f32, space="PSUM", tag="a")
                    nc.tensor.matmul(out=ps2[:], lhsT=h1w[:], rhs=W2_sb[:],
                                     start=True, stop=True)
                    t2 = ep.tile([128, HID], bf, tag="t2")
                    nc.vector.tensor_scalar(out=t2[:], in0=ps2[:],
                                            scalar1=invs_sb[:, w:w + 1],
                                            scalar2=None,
                                            op0=mybir.AluOpType.mult)
                    nc.sync.dma_start(tbl_own[1][w * 128:(w + 1) * 128, :], t2[:])
                else:
                    # h2 window -> transpose -> pooled one-hot accumulate
                    h2w = ep.tile([128, 128], f32, tag="h2w")
                    nc.scalar.activation(h2w[:], ps_w[:],
                                         mybir.ActivationFunctionType.Relu,
                                         bias=b2_sb[:, 0:1])
                    pst = psa.tile([128, 128], f32, space="PSUM", tag="a")
                    nc.tensor.transpose(out=pst[:], in_=h2w[:], identity=ident[:])
                    h2t = ep.tile([128, 128], bf, tag="h2t")
                    nc.vector.tensor_copy(out=h2t[:], in_=pst[:])
                    sg = sp.tile([128, G], bf, tag="sg")
                    nc.vector.tensor_scalar(
                        out=sg[:], in0=iotag_sb[:],
                        scalar1=batch_sb[:, w:w + 1], scalar2=None,
                        op0=mybir.AluOpType.is_equal)
                    nc.tensor.matmul(out=ps_pool[:], lhsT=h2t[:], rhs=sg[:],
                                     start=(w == 0), stop=(w == NW - 1),
                                     skip_group_check=True)

            agg_layer(0)
            if no_coll:
                nc.sync.dma_start(tbl_full[1][:cfg.PAD_NPC, :], tbl_own[1][:])
            else:
                nc.gpsimd.collective_compute(
                    "AllGather", mybir.AluOpType.bypass,
                    ins=[tbl_own[1][:]], outs=[tbl_full[1][:]],
                    replica_groups=replica)
            agg_layer(1)

            # ---- pooled mean + head (replicated on every core)
            pooled_l = ep.tile([128, G], f32, tag="pool")
            nc.vector.tensor_copy(out=pooled_l[:], in_=ps_pool[:])
            ar_in = dp.tile([128, G], f32, tag="arin")
            ar_out = dp.tile([128, G], f32, addr_space="Shared", tag="arout")
            nc.sync.dma_start(ar_in[:], pooled_l[:])
            if no_coll:
                nc.sync.dma_start(ar_out[:], ar_in[:])
            else:
                nc.gpsimd.collective_compute(
                    "AllReduce", mybir.AluOpType.add,
                    ins=[ar_in.opt()], outs=[ar_out.opt()],
                    replica_groups=replica)
            pooled = ep.tile([128, G], f32, tag="pool")
            nc.sync.dma_start(pooled[:], ar_out[:])

            psc = psg.tile([128, G], f32, space="PSUM", tag="g")
            nc.tensor.matmul(out=psc[:], lhsT=ones1_sb[:], rhs=cnt_sb[:],
                             start=True, stop=True)
            cntb = ep.tile([128, G], f32, tag="pool")
            nc.vector.tensor_scalar_max(out=cntb[:], in0=psc[:], scalar1=1.0)
            invc = ep.tile([128, G], f32, tag="pool")
            nc.vector.reciprocal(invc[:], cntb[:])
            pmean = ep.tile([128, G], f32, tag="pool")
            nc.vector.tensor_tensor(out=pmean[:], in0=pooled[:], in1=invc[:],
                                    op=mybir.AluOpType.mult)

            psh1 = psg.tile([128, G], f32, space="PSUM", tag="g")
            nc.tensor.matmul(out=psh1[:], lhsT=Wh1_sb[:], rhs=pmean[:],
                             start=True, stop=True)
            relu1 = ep.tile([128, G], f32, tag="pool")
            nc.scalar.activation(relu1[:], psh1[:],
                                 mybir.ActivationFunctionType.Relu,
                                 bias=bh1_sb[:, 0:1])
            psh2 = psg.tile([cfg.OUT, G], f32, space="PSUM", tag="g")
            nc.tensor.matmul(out=psh2[:], lhsT=Wh2_sb[:], rhs=relu1[:],
                             start=True, stop=True)
            out_sb = ep.tile([cfg.OUT, G], f32, tag="out")
            nc.vector.tensor_scalar_add(out=out_sb[:], in0=psh2[:],
                                        scalar1=bh2_sb[:, 0:1])
            nc.sync.dma_start(t_out[:], out_sb[:])

    nc.compile()
    return nc


# ----------------------------------------------------------------- entry
def _run(inputs, cfg=CFG, trace=False):
    in_maps, meta = _host_prep(cfg, inputs["x"], inputs["edge_index"],
                               inputs["batch"])
    _add_weights(cfg, in_maps,
                 inputs["W1"], inputs["b1"], inputs["W2"], inputs["b2"],
                 inputs["Wh1"], inputs["bh1"], inputs["Wh2"], inputs["bh2"])
    nc = _build(cfg, meta)
    res = run_bass_kernel_spmd(nc, in_maps, list(range(cfg.C)), trace=trace)
    out = np.ascontiguousarray(np.asarray(res.results[0]["out"]).T)
    return out, res


def kernel(**inputs) -> np.ndarray:
    out, _ = _run(inputs, CFG, trace=False)
    return out
